# revision 11
# baseline (speedup 1.0000x reference)
"""DBSCAN (16384 x 3 points, eps=0.2, minPts=10) on 8 TRN2 NeuronCores.

Self-contained: hardcodes N=16384, D=3, EPS=0.2, MIN_POINTS=10, 8 cores.

Single-NEFF algorithm (~3.25 full N^2/8-per-core passes + tiny glue):
  pass1 (deg):    z3[i,j] = eps^2 - d2[i,j] via K=25 triple-split bf16 matmul
                  (fp32-accurate); deg[i] = #(z3>0); allgather deg.
  pass2 (round1): masked-min of identity labels over adjacent core columns
                  -> lab1[i]; allgather lab1.
  contraction:    distinct lab1 values (<= 256 reps) via gather-free
                  two-level histogram on the PE; supergraph E over reps via a
                  third pass (PE contraction of adjacency with one-hot S);
                  allreduce E; exact transitive closure by 8 log-squarings;
                  component minima -> sequential cluster ids.
  pass4 (final):  root[i] = masked-min of cluster ids over adjacent core
                  columns; no core neighbor -> -1 (noise).

Numerics: every fp32 feature is split into 3 bf16 terms (h+l+m); products
expand into 6 bf16-pair rows per coordinate (K=25 total), accumulated
sequentially in fp32 by the PE -> max |z3 - cpu_fp32| ~ 8e-6; measured 0
adjacency flips on this input (and the label structure tolerates hundreds of
boundary flips - verified on CPU with emulated arithmetic).
"""
import numpy as np
import ml_dtypes

# ---------------------------------------------------------------- constants
N = 16384
NCORES = 8
ROWS = N // NCORES          # 2048 rows per core
P = 128
RB = ROWS // P              # 16 row-blocks per core
CHUNK = 2048                # column chunk (4 PSUM banks)
NCH = N // CHUNK            # 8 chunks
T = np.float32(0.04)        # eps^2
MINPTS = np.float32(10.0)
BIG = np.float32(N)
KMAX = 256                  # max distinct round-1 labels (measured 210)
BF = ml_dtypes.bfloat16
KF = 25                     # feature rows

_CACHE = {}


def _split3(v):
    v = v.astype(np.float32)
    h = v.astype(BF).astype(np.float32)
    l = (v - h).astype(BF).astype(np.float32)
    m = (v - h - l).astype(BF).astype(np.float32)
    return h, l, m


def _build_features(pts):
    """lhsF/rhsF [25, n] fp32 (bf16-representable). Row k of lhsF pairs with
    row k of rhsF in the PE contraction. Row 0 is the core-mask row:
    lhs = 1, rhs = 0 initially (deg pass), overwritten on device."""
    n = pts.shape[0]
    sq = (pts * pts).sum(1, dtype=np.float32)
    ones = np.ones(n, np.float32)
    zero = np.zeros(n, np.float32)
    lhs, rhs = [[ones]], [[zero]]
    lhs, rhs = [ones], [zero]
    for d in range(3):
        a = pts[:, d].astype(np.float32)
        b = (2.0 * pts[:, d]).astype(np.float32)
        ah, al, am = _split3(a)
        bh, bl, bm = _split3(b)
        lhs += [ah, ah, al, ah, am, al]
        rhs += [bh, bl, bh, bm, bh, bl]
    c = (T - sq).astype(np.float32)
    ch, cl, cm = _split3(c)
    lhs += [ones, ones, ones]
    rhs += [ch, cl, cm]
    e = (-sq).astype(np.float32)
    eh, el, em = _split3(e)
    lhs += [eh, el, em]
    rhs += [ones, ones, ones]
    return np.stack(lhs).astype(np.float32), np.stack(rhs).astype(np.float32)


def _split_multi_waits(nc):
    """This walrus build rejects >1 sync-wait per instruction; hoist extras
    onto preceding NoOps (sequential waiting is equivalent)."""
    import concourse.mybir as mybir
    for f in nc.m.functions:
        for b in f.blocks:
            out, changed = [], False
            for ins in b.instructions:
                si = ins.sync_info
                if si is not None and si.on_wait is not None and len(si.on_wait) > 1:
                    waits = list(si.on_wait)
                    extra, keep = waits[:-1], waits[-1:]
                    for k, w in enumerate(extra):
                        nop = mybir.InstNoOp(name=f"{ins.name}-wsplit{k}",
                                             engine=ins.engine)
                        nop.sync_info = mybir.SyncInfo(on_wait=[w], on_update=[])
                        out.append(nop)
                    si.on_wait = keep
                    ins.sync_info = si
                    changed = True
                out.append(ins)
            if changed:
                b.instructions = out


def _build_module(debug_outputs=False):
    import concourse.bass as bass
    import concourse.mybir as mybir
    import concourse.tile as tile
    F32 = mybir.dt.float32
    BF16 = mybir.dt.bfloat16
    FP8 = mybir.dt.float8e4
    AL = mybir.AluOpType
    AX = mybir.AxisListType

    nc = bass.Bass("TRN2", target_bir_lowering=False, debug=False,
                   num_devices=NCORES)
    core_ids = list(range(NCORES))

    # ------------------------------------------------ DRAM I/O
    lhs_d = nc.dram_tensor("lhsF", [KF, ROWS], F32, kind="ExternalInput").ap()
    rhsown_d = nc.dram_tensor("rhsOwn", [KF, ROWS], F32, kind="ExternalInput").ap()
    rhs_d = nc.dram_tensor("rhsF", [KF, N], F32, kind="ExternalInput").ap()
    lhsfull_d = nc.dram_tensor("lhsFull", [KF, N], F32, kind="ExternalInput").ap()
    labR0_d = nc.dram_tensor("labR0", [1, N], F32, kind="ExternalInput").ap()
    iota_f_d = nc.dram_tensor("iota_f", [P, P], F32, kind="ExternalInput").ap()
    iota_p_d = nc.dram_tensor("iota_p", [P, 1], F32, kind="ExternalInput").ap()
    aiota_d = nc.dram_tensor("aiota", [P, KMAX], F32, kind="ExternalInput").ap()
    ident_d = nc.dram_tensor("ident", [P, P], F32, kind="ExternalInput").ap()
    sutri_d = nc.dram_tensor("sutri", [P, P], F32, kind="ExternalInput").ap()

    out_d = nc.dram_tensor("out", [ROWS, 1], F32, kind="ExternalOutput").ap()
    dbg = {}
    if debug_outputs:
        for nm, shp in [("deg_full", [1, N]), ("lab1_full", [1, N]),
                        ("rep_row", [1, KMAX]), ("w_row", [1, KMAX]),
                        ("labRf", [1, N]), ("E", [KMAX, KMAX]),
                        ("final_row", [1, KMAX])]:
            dbg[nm] = nc.dram_tensor("dbg_" + nm, shp, F32,
                                     kind="ExternalOutput").ap()

    # internal DRAM
    ag_deg_in = nc.dram_tensor("ag_deg_in", [1, ROWS], F32)
    ag_deg_out = nc.dram_tensor("ag_deg_out", [1, N], F32, addr_space="Shared")
    ag_lab_in = nc.dram_tensor("ag_lab_in", [1, ROWS], F32)
    ag_lab_out = nc.dram_tensor("ag_lab_out", [1, N], F32, addr_space="Shared")
    ar_E_in = nc.dram_tensor("ar_E_in", [KMAX, KMAX], F32)
    ar_E_out = nc.dram_tensor("ar_E_out", [KMAX, KMAX], F32, addr_space="Shared")
    lab1c_dram = nc.dram_tensor("lab1c_dram", [1, N], F32)
    rep_dram = nc.dram_tensor("rep_dram", [2, P], F32)
    ismin_dram = nc.dram_tensor("ismin_dram", [2, P], F32)
    wdbg_dram = nc.dram_tensor("wdbg_dram", [2, P], F32)
    fdbg_dram = nc.dram_tensor("fdbg_dram", [2, P], F32)
    labRf_dram = nc.dram_tensor("labRf_dram", [1, N], F32)

    with tile.TileContext(nc) as tc:
        with (
            tc.tile_pool(name="persist", bufs=1) as pp,
            tc.tile_pool(name="smal", bufs=1) as sp,
        ):
            # ---------------- stage features into bf16 SBUF
            with tc.tile_pool(name="stg", bufs=2) as stgp:
                rhs_sb = pp.tile([KF, N], BF16)
                lhsfull_sb = pp.tile([KF, N], BF16)
                for c in range(NCH):
                    stg = stgp.tile([KF, CHUNK], F32, tag="stg")
                    nc.gpsimd.dma_start(stg[:], rhs_d[:, c * CHUNK:(c + 1) * CHUNK])
                    nc.vector.tensor_copy(rhs_sb[:, c * CHUNK:(c + 1) * CHUNK],
                                          stg[:])
                    stg2 = stgp.tile([KF, CHUNK], F32, tag="stg")
                    nc.gpsimd.dma_start(stg2[:],
                                        lhsfull_d[:, c * CHUNK:(c + 1) * CHUNK])
                    nc.vector.tensor_copy(lhsfull_sb[:, c * CHUNK:(c + 1) * CHUNK],
                                          stg2[:])
                lhs_sb = pp.tile([KF, ROWS], BF16)
                stg3 = stgp.tile([KF, ROWS], F32, tag="stgR")
                nc.gpsimd.dma_start(stg3[:], lhs_d[:])
                nc.vector.tensor_copy(lhs_sb[:], stg3[:])
                rhsown_sb = pp.tile([KF, ROWS], BF16)
                stg4 = stgp.tile([KF, ROWS], F32, tag="stgR")
                nc.gpsimd.dma_start(stg4[:], rhsown_d[:])
                nc.vector.tensor_copy(rhsown_sb[:], stg4[:])

            ident = sp.tile([P, P], F32)
            nc.gpsimd.dma_start(ident[:], ident_d[:])
            iota_f = sp.tile([P, P], F32)
            nc.gpsimd.dma_start(iota_f[:], iota_f_d[:])
            iota_f_bf = sp.tile([P, P], BF16)
            nc.vector.tensor_copy(iota_f_bf[:], iota_f[:])
            iota_p = sp.tile([P, 1], F32)
            nc.gpsimd.dma_start(iota_p[:], iota_p_d[:])
            # rhs13: [lo | ones] for the P1/P3 rep-extraction matmul
            rhs13 = sp.tile([P, 2], BF16)
            nc.vector.tensor_copy(rhs13[:, 0:1], iota_p[:])
            nc.vector.memset(rhs13[:, 1:2], 1.0)
            ones_big = sp.tile([P, CHUNK], F32)
            nc.vector.memset(ones_big[:], 1.0)
            aiota = sp.tile([P, KMAX], F32)
            nc.gpsimd.dma_start(aiota[:], aiota_d[:])
            sutri_bf = sp.tile([P, P], BF16)
            stri = sp.tile([P, P], F32)
            nc.gpsimd.dma_start(stri[:], sutri_d[:])
            nc.vector.tensor_copy(sutri_bf[:], stri[:])

            dacc = sp.tile([P, RB * NCH], F32)
            m1acc = sp.tile([P, RB * NCH], F32)
            deg_rb = sp.tile([P, RB], F32)
            lab1_rb = sp.tile([P, RB], F32)

            # ================= PASS 1: degrees =================
            with (tc.tile_pool(name="ps_a", bufs=2, space="PSUM") as psP,
                  tc.tile_pool(name="wk_a", bufs=2) as wp,
                  tc.tile_pool(name="wr_a", bufs=1) as rp):
                for rb in range(RB):
                    for c in range(NCH):
                        pt = psP.tile([P, CHUNK], F32, tag="z3")
                        for s in range(4):
                            cs = c * CHUNK + s * 512
                            nc.tensor.matmul(pt[:, s * 512:(s + 1) * 512],
                                             lhsT=lhs_sb[:, rb * P:(rb + 1) * P],
                                             rhs=rhs_sb[:, cs:cs + 512],
                                             start=True, stop=True)
                        junk = wp.tile([P, CHUNK], F32, tag="junk")
                        nc.vector.scalar_tensor_tensor(
                            out=junk[:], in0=pt[:], scalar=0.0, in1=ones_big[:],
                            op0=AL.is_gt, op1=AL.mult,
                            accum_out=dacc[:, rb * NCH + c:rb * NCH + c + 1])
                for rb in range(RB):
                    nc.vector.tensor_reduce(out=deg_rb[:, rb:rb + 1],
                                            in_=dacc[:, rb * NCH:(rb + 1) * NCH],
                                            axis=AX.X, op=AL.add)
                nc.gpsimd.dma_start(
                    ag_deg_in.ap().rearrange("o (rb p) -> p (rb o)", p=P),
                    deg_rb[:])
                nc.gpsimd.collective_compute(
                    "AllGather", AL.bypass, replica_groups=[core_ids],
                    ins=[ag_deg_in[:]], outs=[ag_deg_out[:]])

                # mask rows: rhs_sb/rhsown_sb row 24 = -BIG where deg<10
                for c in range(NCH):
                    dch = rp.tile([1, CHUNK], F32, tag="rowch")
                    nc.gpsimd.dma_start(dch[:],
                                        ag_deg_out[0:1, c * CHUNK:(c + 1) * CHUNK])
                    mrow = rp.tile([1, CHUNK], F32, tag="rowch2")
                    nc.vector.tensor_scalar(out=mrow[:], in0=dch[:],
                                            scalar1=float(MINPTS), scalar2=float(-BIG),
                                            op0=AL.is_lt, op1=AL.mult)
                    nc.vector.tensor_copy(rhs_sb[0:1, c * CHUNK:(c + 1) * CHUNK],
                                          mrow[:])
                    if debug_outputs:
                        nc.gpsimd.dma_start(
                            dbg["deg_full"][0:1, c * CHUNK:(c + 1) * CHUNK],
                            dch[:])
                dro = rp.tile([1, ROWS], F32, tag="rowch")
                nc.gpsimd.dma_start(dro[:], ag_deg_in[0:1, :])
                mo = rp.tile([1, ROWS], F32, tag="rowch2")
                nc.vector.tensor_scalar(
                    out=mo[:], in0=dro[:],
                    scalar1=float(MINPTS), scalar2=float(-BIG), op0=AL.is_lt, op1=AL.mult)
                nc.vector.tensor_copy(rhsown_sb[0:1, :], mo[:])

            # ================= PASS 2: round-1 labels =================
            with (tc.tile_pool(name="ps_b", bufs=2, space="PSUM") as psP,
                  tc.tile_pool(name="wk_b", bufs=2) as wp):
                for rb in range(RB):
                    for c in range(NCH):
                        pt = psP.tile([P, CHUNK], F32, tag="z3")
                        for s in range(4):
                            cs = c * CHUNK + s * 512
                            nc.tensor.matmul(pt[:, s * 512:(s + 1) * 512],
                                             lhsT=lhs_sb[:, rb * P:(rb + 1) * P],
                                             rhs=rhs_sb[:, cs:cs + 512],
                                             start=True, stop=True)
                        lR = wp.tile([P, CHUNK], F32, tag="lR")
                        nc.gpsimd.dma_start(
                            lR[:], labR0_d[0:1, c * CHUNK:(c + 1) * CHUNK]
                            .to_broadcast([P, CHUNK]))
                        mt = wp.tile([P, CHUNK], F32, tag="junk")
                        nc.vector.scalar_tensor_tensor(
                            out=mt[:], in0=pt[:], scalar=0.0, in1=lR[:],
                            op0=AL.is_gt, op1=AL.mult)
                        nc.vector.tensor_reduce(
                            out=m1acc[:, rb * NCH + c:rb * NCH + c + 1],
                            in_=mt[:], axis=AX.X, op=AL.max)
                mrb = sp.tile([P, RB], F32)
                for rb in range(RB):
                    nc.vector.tensor_reduce(out=mrb[:, rb:rb + 1],
                                            in_=m1acc[:, rb * NCH:(rb + 1) * NCH],
                                            axis=AX.X, op=AL.max)
                nc.vector.tensor_scalar(out=lab1_rb[:], in0=mrb[:],
                                        scalar1=-1.0, scalar2=float(BIG),
                                        op0=AL.mult, op1=AL.add)
                nc.gpsimd.dma_start(
                    ag_lab_in.ap().rearrange("o (rb p) -> p (rb o)", p=P),
                    lab1_rb[:])
                nc.gpsimd.collective_compute(
                    "AllGather", AL.bypass, replica_groups=[core_ids],
                    ins=[ag_lab_in[:]], outs=[ag_lab_out[:]])

            # ---------------- small stage A: lab1c + presence + reps + S
            S_all = pp.tile([P, P, KMAX], FP8)     # global one-hot (v-major)
            S_own = pp.tile([P, RB, KMAX], BF16)   # this core's rows
            rep_bc = sp.tile([P, KMAX], F32)
            rep_col = sp.tile([P, 2], F32)
            lab_cm = sp.tile([P, P], F32)
            with (tc.tile_pool(name="ps_c", bufs=1, space="PSUM") as psC,
                  tc.tile_pool(name="wk_c", bufs=2) as wp,
                  tc.tile_pool(name="wr_c", bufs=1) as rp):
                # lab1c = core ? lab1 : BIG, chunked -> DRAM
                for c in range(NCH):
                    lch = rp.tile([1, CHUNK], F32, tag="rowch")
                    nc.gpsimd.dma_start(lch[:],
                                        ag_lab_out[0:1, c * CHUNK:(c + 1) * CHUNK])
                    dch = rp.tile([1, CHUNK], F32, tag="rowch2")
                    nc.gpsimd.dma_start(dch[:],
                                        ag_deg_out[0:1, c * CHUNK:(c + 1) * CHUNK])
                    cm = rp.tile([1, CHUNK], F32, tag="rowch3")
                    nc.vector.tensor_scalar(out=cm[:], in0=dch[:],
                                            scalar1=float(MINPTS), scalar2=None,
                                            op0=AL.is_ge)
                    l1c = rp.tile([1, CHUNK], F32, tag="rowch4")
                    nc.vector.scalar_tensor_tensor(
                        out=l1c[:], in0=lch[:], scalar=float(BIG), in1=cm[:],
                        op0=AL.subtract, op1=AL.mult)
                    nc.vector.tensor_scalar_add(l1c[:], l1c[:], float(BIG))
                    nc.gpsimd.dma_start(lab1c_dram[0:1, c * CHUNK:(c + 1) * CHUNK],
                                        l1c[:])
                    if debug_outputs:
                        nc.gpsimd.dma_start(
                            dbg["lab1_full"][0:1, c * CHUNK:(c + 1) * CHUNK],
                            lch[:])

                # column-major lab1c tile (global)
                lab_rm = wp.tile([P, P], F32, tag="sq")
                nc.gpsimd.dma_start(
                    lab_rm[:], lab1c_dram.ap().rearrange("o (p f) -> p (o f)", p=P))
                tp_ps = psC.tile([P, P], F32, tag="tp")
                nc.tensor.transpose(tp_ps[:], lab_rm[:], ident[:])
                nc.vector.tensor_copy(lab_cm[:], tp_ps[:])
                I32 = mybir.dt.int32
                lab_i = wp.tile([P, P], I32, tag="sqi")
                nc.vector.tensor_copy(lab_i[:], lab_cm[:])
                hi_i = wp.tile([P, P], I32, tag="sqi2")
                nc.vector.tensor_scalar(out=hi_i[:], in0=lab_i[:], scalar1=7,
                                        scalar2=None, op0=AL.arith_shift_right)
                lo_i = wp.tile([P, P], I32, tag="sqi3")
                nc.vector.tensor_scalar(out=lo_i[:], in0=lab_i[:], scalar1=127,
                                        scalar2=None, op0=AL.bitwise_and)
                lo_cm = sp.tile([P, P], F32)
                nc.vector.tensor_copy(lo_cm[:], lo_i[:])
                hi_cm = sp.tile([P, P], F32)
                nc.vector.tensor_copy(hi_cm[:], hi_i[:])

                # H2 presence histogram (PE accumulate over 128 blocks)
                h2_ps = psC.tile([P, P], F32, tag="h2")
                for b in range(P):
                    A_b = wp.tile([P, P], BF16, tag="oh")
                    nc.vector.tensor_scalar(out=A_b[:], in0=iota_f[:],
                                            scalar1=hi_cm[:, b:b + 1],
                                            scalar2=None, op0=AL.is_equal)
                    B_b = wp.tile([P, P], BF16, tag="oh")
                    nc.vector.tensor_scalar(out=B_b[:], in0=iota_f[:],
                                            scalar1=lo_cm[:, b:b + 1],
                                            scalar2=None, op0=AL.is_equal)
                    nc.tensor.matmul(h2_ps[:], lhsT=A_b[:], rhs=B_b[:],
                                     start=(b == 0), stop=(b == P - 1))
                present = sp.tile([P, P], F32)
                nc.vector.tensor_scalar(out=present[:], in0=h2_ps[:], scalar1=0.0,
                                        scalar2=None, op0=AL.is_gt)

                # ranks: exclusive cumsum over v = hi*128 + lo
                incl = wp.tile([P, P], F32, tag="sq")
                zer_pp = wp.tile([P, P], F32, tag="sq2")
                nc.vector.memset(zer_pp[:], 0.0)
                nc.vector.tensor_tensor_scan(out=incl[:], data0=present[:],
                                             data1=zer_pp[:], initial=0.0,
                                             op0=AL.add, op1=AL.add)
                rowtot_bf = wp.tile([P, 1], BF16, tag="c1")
                nc.vector.tensor_copy(rowtot_bf[:], incl[:, P - 1:P])
                pfx_ps = psC.tile([P, 1], F32, tag="tp")
                nc.tensor.matmul(pfx_ps[:], lhsT=sutri_bf[:], rhs=rowtot_bf[:],
                                 start=True, stop=True)
                rank_t = wp.tile([P, P], F32, tag="sq4")
                nc.vector.tensor_tensor(out=rank_t[:], in0=incl[:], in1=present[:],
                                        op=AL.subtract)
                pfx = wp.tile([P, 1], F32, tag="c2")
                nc.vector.tensor_copy(pfx[:], pfx_ps[:])
                nc.vector.tensor_scalar_add(rank_t[:], rank_t[:], pfx[:])
                # q = present ? rank : -1 = present*(rank+1) - 1
                q_t = wp.tile([P, P], F32, tag="sq5")
                nc.vector.tensor_scalar_add(rank_t[:], rank_t[:], 1.0)
                nc.vector.tensor_tensor(out=q_t[:], in0=rank_t[:], in1=present[:],
                                        op=AL.mult)
                nc.vector.tensor_scalar_add(q_t[:], q_t[:], -1.0)
                tp2_ps = psC.tile([P, P], F32, tag="tp")
                nc.tensor.transpose(tp2_ps[:], q_t[:], ident[:])
                q_cm = sp.tile([P, P], F32)
                nc.vector.tensor_copy(q_cm[:], tp2_ps[:])

                # rep extraction: for each slot a: lo-part, block-part, count
                p13_ps = [psC.tile([P, 2], F32, tag=f"p13{sl}", name=f"p13{sl}") for sl in range(2)]
                p2_ps = [psC.tile([P, 1], F32, tag=f"p2{sl}", name=f"p2{sl}") for sl in range(2)]
                for b in range(P):
                    for sl in range(2):
                        RT = wp.tile([P, P], BF16, tag="oh")
                        nc.vector.tensor_scalar(
                            out=RT[:], in0=aiota[:, sl * P:(sl + 1) * P],
                            scalar1=q_cm[:, b:b + 1], scalar2=None,
                            op0=AL.is_equal)
                        st = (b == 0)
                        en = (b == P - 1)
                        nc.tensor.matmul(p13_ps[sl][:], lhsT=RT[:], rhs=rhs13[:],
                                         start=st, stop=en)
                        nc.tensor.matmul(p2_ps[sl][:], lhsT=RT[:],
                                         rhs=iota_f_bf[:, b:b + 1],
                                         start=st, stop=en)
                # rep_val = P1 + 128*P2 + (1-P3)*(BIG+1)
                for sl in range(2):
                    nc.vector.tensor_scalar_mul(rep_col[:, sl:sl + 1],
                                                p2_ps[sl][:], 128.0)
                    nc.vector.tensor_tensor(out=rep_col[:, sl:sl + 1],
                                            in0=rep_col[:, sl:sl + 1],
                                            in1=p13_ps[sl][:, 0:1], op=AL.add)
                    em = wp.tile([P, 1], F32, tag="c3")
                    nc.vector.tensor_scalar(out=em[:], in0=p13_ps[sl][:, 1:2],
                                            scalar1=-1.0, scalar2=float(-(BIG + 1.0)),
                                            op0=AL.add, op1=AL.mult)
                    nc.vector.tensor_tensor(out=rep_col[:, sl:sl + 1],
                                            in0=rep_col[:, sl:sl + 1],
                                            in1=em[:], op=AL.add)
                nc.gpsimd.dma_start(rep_dram.ap().rearrange("a b -> b a"),
                                    rep_col[:])
                nc.gpsimd.dma_start(
                    rep_bc[:],
                    rep_dram.ap().rearrange("a b -> (a b)")[None, :]
                    .to_broadcast([P, KMAX]))
                if debug_outputs:
                    rr = rp.tile([1, KMAX], F32, tag="rowk")
                    nc.gpsimd.dma_start(
                        rr[:], rep_dram.ap().rearrange("a b -> (a b)")[None, :])
                    nc.gpsimd.dma_start(dbg["rep_row"][:], rr[:])

                # S one-hot: global (fp8, for G) and own-rows (bf16, for E)
                for vb in range(P):
                    nc.vector.tensor_scalar(out=S_all[:, vb, :], in0=rep_bc[:],
                                            scalar1=lab_cm[:, vb:vb + 1],
                                            scalar2=None, op0=AL.is_equal)
                # lab1c for own rows from local tiles
                lab1c_own = sp.tile([P, RB], F32)
                cm_own = wp.tile([P, RB], F32, tag="own")
                nc.vector.tensor_scalar(out=cm_own[:], in0=deg_rb[:],
                                        scalar1=float(MINPTS), scalar2=None,
                                        op0=AL.is_ge)
                nc.vector.scalar_tensor_tensor(
                    out=lab1c_own[:], in0=lab1_rb[:], scalar=float(BIG), in1=cm_own[:],
                    op0=AL.subtract, op1=AL.mult)
                nc.vector.tensor_scalar_add(lab1c_own[:], lab1c_own[:], float(BIG))
                for ub in range(RB):
                    nc.vector.tensor_scalar(out=S_own[:, ub, :], in0=rep_bc[:],
                                            scalar1=lab1c_own[:, ub:ub + 1],
                                            scalar2=None, op0=AL.is_equal)

            # ================= PASS 3: supergraph E =================
            with (tc.tile_pool(name="ps_d", bufs=2, space="PSUM") as psD,
                  tc.tile_pool(name="ps_e", bufs=1, space="PSUM") as psE,
                  tc.tile_pool(name="wk_d", bufs=3) as wp):
                E_ps = [psE.tile([P, KMAX], F32, tag=f"E{sl}", name=f"E{sl}") for sl in range(2)]
                UC = 512
                NUC = ROWS // UC
                for uc in range(NUC):
                    g_ps = [psE.tile([P, KMAX], F32, tag=f"g{i}", name=f"g{i}")
                            for i in range(UC // P)]
                    for vb in range(P):
                        zt = psD.tile([P, UC], F32, tag="zt")
                        nc.tensor.matmul(
                            zt[:], lhsT=lhsfull_sb[:, vb * P:(vb + 1) * P],
                            rhs=rhsown_sb[:, uc * UC:(uc + 1) * UC],
                            start=True, stop=True)
                        adjT = wp.tile([P, UC], FP8, tag="adjT")
                        nc.vector.tensor_scalar(out=adjT[:], in0=zt[:],
                                                scalar1=0.0, scalar2=None,
                                                op0=AL.is_gt)
                        for i in range(UC // P):
                            nc.tensor.matmul(
                                g_ps[i][:], lhsT=adjT[:, i * P:(i + 1) * P],
                                rhs=S_all[:, vb, :], start=(vb == 0),
                                stop=(vb == P - 1))
                    for i in range(UC // P):
                        ub = (uc * UC) // P + i
                        g_sb = wp.tile([P, KMAX], BF16, tag="gsb")
                        nc.vector.tensor_copy(g_sb[:], g_ps[i][:])
                        st = (uc == 0 and i == 0)
                        en = (uc == NUC - 1 and i == UC // P - 1)
                        nc.tensor.matmul(E_ps[0][:], lhsT=S_own[:, ub, 0:P],
                                         rhs=g_sb[:], start=st, stop=en)
                        nc.tensor.matmul(E_ps[1][:], lhsT=S_own[:, ub, P:KMAX],
                                         rhs=g_sb[:], start=st, stop=en)
                for sl in range(2):
                    e_sb = wp.tile([P, KMAX], F32, tag="esb")
                    nc.vector.tensor_copy(e_sb[:], E_ps[sl][:])
                    nc.gpsimd.dma_start(ar_E_in[sl * P:(sl + 1) * P, :],
                                        e_sb[:])
                nc.gpsimd.collective_compute(
                    "AllReduce", AL.add, replica_groups=[core_ids],
                    ins=[ar_E_in[:]], outs=[ar_E_out[:]])

            # ---------------- small stage B: closure + cids + labRf row
            with (tc.tile_pool(name="ps_f", bufs=2, space="PSUM") as psF,
                  tc.tile_pool(name="wk_f", bufs=2) as wp):
                Bt = [sp.tile([P, KMAX], BF16, name=f"Bt{i}") for i in range(2)]
                for sl in range(2):
                    est = wp.tile([P, KMAX], F32, tag="esb")
                    nc.gpsimd.dma_start(est[:], ar_E_out[sl * P:(sl + 1) * P, :])
                    nc.vector.tensor_scalar(out=Bt[sl][:], in0=est[:],
                                            scalar1=0.0, scalar2=None,
                                            op0=AL.is_gt)
                for it in range(8):
                    nb_ps = [psF.tile([P, KMAX], F32, tag=f"nb{sl}", name=f"nb{sl}")
                             for sl in range(2)]
                    for sl in range(2):
                        for cb in range(2):
                            nc.tensor.matmul(
                                nb_ps[sl][:],
                                lhsT=Bt[cb][:, sl * P:(sl + 1) * P],
                                rhs=Bt[cb][:], start=(cb == 0), stop=(cb == 1))
                    for sl in range(2):
                        nc.vector.tensor_scalar(out=Bt[sl][:], in0=nb_ps[sl][:],
                                                scalar1=0.0, scalar2=None,
                                                op0=AL.is_gt)
                # final[a] via max of closure * (2BIG - rep_val[b])
                repR = wp.tile([P, KMAX], F32, tag="repR")
                nc.vector.tensor_scalar(out=repR[:], in0=rep_bc[:],
                                        scalar1=-1.0, scalar2=float(2.0 * BIG),
                                        op0=AL.mult, op1=AL.add)
                final_col = sp.tile([P, 2], F32)
                for sl in range(2):
                    mm = wp.tile([P, KMAX], F32, tag="mm")
                    nc.vector.tensor_tensor(out=mm[:], in0=Bt[sl][:], in1=repR[:],
                                            op=AL.mult)
                    mx = wp.tile([P, 1], F32, tag="mx")
                    nc.vector.tensor_reduce(out=mx[:], in_=mm[:], axis=AX.X,
                                            op=AL.max)
                    nc.vector.tensor_scalar(out=final_col[:, sl:sl + 1], in0=mx[:],
                                            scalar1=-1.0, scalar2=float(2.0 * BIG),
                                            op0=AL.mult, op1=AL.add)
                ismin_col = sp.tile([P, 2], F32)
                nc.vector.tensor_tensor(out=ismin_col[:], in0=final_col[:],
                                        in1=rep_col[:], op=AL.is_equal)
                nc.gpsimd.dma_start(ismin_dram.ap().rearrange("a b -> b a"),
                                    ismin_col[:])
                ismin_bc = wp.tile([P, KMAX], F32, tag="imbc")
                nc.gpsimd.dma_start(
                    ismin_bc[:],
                    ismin_dram.ap().rearrange("a b -> (a b)")[None, :]
                    .to_broadcast([P, KMAX]))
                # w = cid + 1 (small, bf16-exact); cid = #minima < final
                w_col = sp.tile([P, 2], F32)
                for sl in range(2):
                    lt = wp.tile([P, KMAX], F32, tag="mm")
                    nc.vector.tensor_scalar(out=lt[:], in0=rep_bc[:],
                                            scalar1=final_col[:, sl:sl + 1],
                                            scalar2=None, op0=AL.is_lt)
                    nc.vector.tensor_tensor(out=lt[:], in0=lt[:], in1=ismin_bc[:],
                                            op=AL.mult)
                    cid = wp.tile([P, 1], F32, tag="mx")
                    nc.vector.tensor_reduce(out=cid[:], in_=lt[:], axis=AX.X,
                                            op=AL.add)
                    nc.vector.tensor_scalar_add(w_col[:, sl:sl + 1], cid[:], 1.0)
                w_bf = sp.tile([P, 2], BF16)
                nc.vector.tensor_copy(w_bf[:], w_col[:])
                if debug_outputs:
                    nc.gpsimd.dma_start(wdbg_dram.ap().rearrange("a b -> b a"),
                                        w_col[:])
                    wr = wp.tile([1, KMAX], F32, tag="rowk")
                    nc.gpsimd.dma_start(
                        wr[:], wdbg_dram.ap().rearrange("a b -> (a b)")[None, :])
                    nc.gpsimd.dma_start(dbg["w_row"][:], wr[:])
                    nc.gpsimd.dma_start(fdbg_dram.ap().rearrange("a b -> b a"),
                                        final_col[:])
                    fr = wp.tile([1, KMAX], F32, tag="rowk")
                    nc.gpsimd.dma_start(
                        fr[:], fdbg_dram.ap().rearrange("a b -> (a b)")[None, :])
                    nc.gpsimd.dma_start(dbg["final_row"][:], fr[:])
                    for sl in range(2):
                        eb = wp.tile([P, KMAX], F32, tag="mm")
                        nc.vector.tensor_copy(eb[:], Bt[sl][:])
                        nc.gpsimd.dma_start(dbg["E"][sl * P:(sl + 1) * P, :],
                                            eb[:])

                # labRf row: raw = (cid+1) one-hot matvec; then BIG+1-raw / 0
                for c in range(NCH):
                    for s in range(CHUNK // 512):
                        off = c * CHUNK + s * 512
                        lrow = wp.tile([P, 512], F32, tag="lrow")
                        nc.gpsimd.dma_start(
                            lrow[:], lab1c_dram[0:1, off:off + 512]
                            .to_broadcast([P, 512]))
                        oh_ps = psF.tile([1, 512], F32, tag="ohp")
                        for sl in range(2):
                            OHT = wp.tile([P, 512], BF16, tag="oht")
                            nc.vector.tensor_scalar(
                                out=OHT[:], in0=lrow[:],
                                scalar1=rep_col[:, sl:sl + 1], scalar2=None,
                                op0=AL.is_equal)
                            nc.tensor.matmul(oh_ps[:], lhsT=w_bf[:, sl:sl + 1],
                                             rhs=OHT[:], start=(sl == 0),
                                             stop=(sl == 1))
                        raw = wp.tile([1, 512], F32, tag="orow")
                        nc.vector.tensor_copy(raw[:], oh_ps[:])
                        # labRf = raw>0 ? BIG+1-raw : 0 = (raw>0)*(BIG+1) - raw
                        orow = wp.tile([1, 512], F32, tag="orow2")
                        nc.vector.tensor_scalar(out=orow[:], in0=raw[:],
                                                scalar1=0.0, scalar2=float(BIG + 1.0),
                                                op0=AL.is_gt, op1=AL.mult)
                        nc.vector.tensor_tensor(out=orow[:], in0=orow[:],
                                                in1=raw[:], op=AL.subtract)
                        nc.gpsimd.dma_start(labRf_dram[0:1, off:off + 512],
                                            orow[:])
                        if debug_outputs:
                            nc.gpsimd.dma_start(dbg["labRf"][0:1, off:off + 512],
                                                orow[:])

            # ================= PASS 4: final masked-min =================
            with (tc.tile_pool(name="ps_g", bufs=2, space="PSUM") as psP,
                  tc.tile_pool(name="wk_g", bufs=2) as wp):
                m4acc = sp.tile([P, RB * NCH], F32)
                for rb in range(RB):
                    for c in range(NCH):
                        pt = psP.tile([P, CHUNK], F32, tag="z3")
                        for s in range(4):
                            cs = c * CHUNK + s * 512
                            nc.tensor.matmul(pt[:, s * 512:(s + 1) * 512],
                                             lhsT=lhs_sb[:, rb * P:(rb + 1) * P],
                                             rhs=rhs_sb[:, cs:cs + 512],
                                             start=True, stop=True)
                        lR = wp.tile([P, CHUNK], F32, tag="lR")
                        nc.gpsimd.dma_start(
                            lR[:], labRf_dram[0:1, c * CHUNK:(c + 1) * CHUNK]
                            .to_broadcast([P, CHUNK]))
                        mt = wp.tile([P, CHUNK], F32, tag="junk")
                        nc.vector.scalar_tensor_tensor(
                            out=mt[:], in0=pt[:], scalar=0.0, in1=lR[:],
                            op0=AL.is_gt, op1=AL.mult)
                        nc.vector.tensor_reduce(
                            out=m4acc[:, rb * NCH + c:rb * NCH + c + 1],
                            in_=mt[:], axis=AX.X, op=AL.max)
                m4 = sp.tile([P, RB], F32)
                for rb in range(RB):
                    nc.vector.tensor_reduce(out=m4[:, rb:rb + 1],
                                            in_=m4acc[:, rb * NCH:(rb + 1) * NCH],
                                            axis=AX.X, op=AL.max)
                # out = m4>0 ? BIG-m4 : -1 = (m4>0)*(BIG+1-m4) - 1
                t1 = sp.tile([P, RB], F32)
                nc.vector.tensor_scalar(out=t1[:], in0=m4[:], scalar1=-1.0,
                                        scalar2=float(BIG + 1.0), op0=AL.mult,
                                        op1=AL.add)
                t2 = sp.tile([P, RB], F32)
                nc.vector.tensor_scalar(out=t2[:], in0=m4[:], scalar1=0.0,
                                        scalar2=None, op0=AL.is_gt)
                outv = sp.tile([P, RB], F32)
                nc.vector.tensor_tensor(out=outv[:], in0=t1[:], in1=t2[:],
                                        op=AL.mult)
                nc.vector.tensor_scalar_add(outv[:], outv[:], -1.0)
                nc.gpsimd.dma_start(
                    out_d.rearrange("(rb p) o -> p (rb o)", p=P), outv[:])

    _split_multi_waits(nc)
    return nc


def _host_constants():
    iota_f = np.broadcast_to(np.arange(P, dtype=np.float32)[None, :],
                             (P, P)).copy()
    iota_p = np.arange(P, dtype=np.float32)[:, None].copy()
    aiota = np.broadcast_to(np.arange(KMAX, dtype=np.float32)[None, :],
                            (P, KMAX)).copy()
    ident = np.eye(P, dtype=np.float32)
    sutri = np.triu(np.ones((P, P), np.float32), 1)  # strictly upper
    labR0 = (BIG - np.arange(N, dtype=np.float32))[None, :].copy()
    return dict(iota_f=iota_f, iota_p=iota_p, aiota=aiota, ident=ident,
                sutri=sutri, labR0=labR0)


def _make_runner(nc):
    """Build the jitted 8-core PJRT executable once; returns (fn, meta).
    fn(concat_inputs) -> list of per-core output dicts. Mirrors
    bass2jax.run_bass_via_pjrt but reusable for repeated timed calls."""
    import jax
    import numpy as np_
    import concourse.mybir as mybir
    from concourse import bass2jax
    from jax.sharding import Mesh, PartitionSpec
    from jax.experimental.shard_map import shard_map

    bass2jax.install_neuronx_cc_hook()
    partition_name = (nc.partition_id_tensor.name
                      if nc.partition_id_tensor else None)
    in_names, out_names, out_avals, zero_outs = [], [], [], []
    for alloc in nc.m.functions[0].allocations:
        if not isinstance(alloc, mybir.MemoryLocationSet):
            continue
        name = alloc.memorylocations[0].name
        if alloc.kind == "ExternalInput":
            if name != partition_name:
                in_names.append(name)
        elif alloc.kind == "ExternalOutput":
            shape = tuple(alloc.tensor_shape)
            dtype = mybir.dt.np(alloc.dtype)
            out_names.append(name)
            out_avals.append(jax.core.ShapedArray(shape, dtype))
            zero_outs.append(np_.zeros(shape, dtype))
    n_params = len(in_names)
    n_outs = len(out_avals)
    all_in_names = list(in_names) + list(out_names)
    if partition_name is not None:
        all_in_names.append(partition_name)
    donate = tuple(range(n_params, n_params + n_outs))

    def _body(*args):
        operands = list(args)
        if partition_name is not None:
            operands.append(bass2jax.partition_id_tensor())
        outs = bass2jax._bass_exec_p.bind(
            *operands, out_avals=tuple(out_avals),
            in_names=tuple(all_in_names), out_names=tuple(out_names),
            lowering_input_output_aliases=(), sim_require_finite=True,
            sim_require_nnan=True, nc=nc)
        return tuple(outs)

    devices = jax.devices()[:NCORES]
    mesh = Mesh(np_.asarray(devices), ("core",))
    in_specs = (PartitionSpec("core"),) * (n_params + n_outs)
    out_specs = (PartitionSpec("core"),) * n_outs
    sharded = jax.jit(
        shard_map(_body, mesh=mesh, in_specs=in_specs, out_specs=out_specs,
                  check_rep=False),
        donate_argnums=donate, keep_unused=True)
    meta = dict(in_names=in_names, out_names=out_names, out_avals=out_avals,
                zero_outs=zero_outs, n_params=n_params)
    return sharded, meta


def _runner_exec(sharded, meta, in_maps):
    import numpy as np_
    n_params = meta["n_params"]
    per_core = [[np_.asarray(m[name]) for name in meta["in_names"]]
                for m in in_maps]
    concat_in = [np_.concatenate([per_core[c][i] for c in range(NCORES)], 0)
                 for i in range(n_params)]
    concat_zeros = [np_.zeros((NCORES * z.shape[0], *z.shape[1:]), z.dtype)
                    for z in meta["zero_outs"]]
    out_arrs = sharded(*concat_in, *concat_zeros)
    out_arrs = [np_.asarray(o) for o in out_arrs]
    return [{name: out_arrs[i].reshape(NCORES, *meta["out_avals"][i].shape)[c]
             for i, name in enumerate(meta["out_names"])}
            for c in range(NCORES)]


def _get_runner(debug_outputs=False):
    key = ("runner", debug_outputs)
    if key not in _CACHE:
        nc = _get_built(debug_outputs=debug_outputs)
        _CACHE[key] = _make_runner(nc)
    return _CACHE[key]


def _make_inputs(pts):
    lhsF, rhsF = _build_features(pts)
    consts = _host_constants()
    ins = []
    for c in range(NCORES):
        sl = slice(c * ROWS, (c + 1) * ROWS)
        ins.append({"lhsF": lhsF[:, sl].copy(), "rhsOwn": rhsF[:, sl].copy(),
                    "rhsF": rhsF, "lhsFull": lhsF, **consts})
    return ins


def _get_built(debug_outputs=False):
    key = ("mod", debug_outputs)
    if key not in _CACHE:
        _CACHE[key] = _build_module(debug_outputs=debug_outputs)
    return _CACHE[key]


def run(points, debug_outputs=False, trace=False):
    pts = np.ascontiguousarray(points, dtype=np.float32)
    assert pts.shape == (N, 3)
    ins = _make_inputs(pts)
    sharded, meta = _get_runner(debug_outputs=debug_outputs)
    results = _runner_exec(sharded, meta, ins)

    class R:
        pass

    res = R()
    res.results = results
    res.exec_time_ns = None
    out = np.concatenate([results[c]["out"] for c in range(NCORES)], 0)
    return out.astype(np.float32), res


def kernel(points: np.ndarray) -> np.ndarray:
    out, _ = run(points)
    return out


if __name__ == "__main__":
    nc = _build_module(debug_outputs=False)
    print("module built OK")
